# revision 37
# baseline (speedup 1.0000x reference)
"""ArcFace (AngularPenaltySMLoss) on 8 TRN2 NeuronCores.

Sharding (model-parallel softmax): 32768 classes split across 8 cores
(4096 each). Host prep is layout/dtype only: fT [512,2048] fp8 (raw),
wT [512,4096] bf16 per shard, fnat/wtgt [2048,512] bf16. No host math.

Per-core structure (v2 — engine-balanced):
  - Weight-col norms: DVE squares -> PE matmul with a ones[128,128]
    stationary, so the column sums land REPLICATED across all 128
    partitions (no separate broadcast matmul). ACT Ln/Exp -> rnr
    [128,c] bf16; DVE scales wT into fp8 whats. Both 4-chunk groups are
    prepped upfront; the two [128,2048] PSUM slots host the row sums
    and are recycled into the main-loop ping-pong.
  - Main loop, class-half-major, j-OUTER per half: the fp8 DoubleRow
    stationary (features) is loaded once per k-pair and reused across
    the 4 class chunks. ACT Exp in place on PSUM with per-partition
    scale 64/||f|| and accum_out row sums.
  - ssf/rawdot/wn2 (= |f|^2, f.wtgt, |wtgt|^2 per row): Pool muls +
    DVE reduces, all ssf first so the Exp scale is ready ahead of the
    sweep.
  - Collectives: AllGather sumsA after sweep A (hidden under sweep B);
    sumsB gathered in two b-halves so only the last 8 b-tiles' flight
    is tail-exposed. Local strided DVE reduce; the combine chain runs
    everything except the final denominator add during the AG flight.
  - All ACT functions forced into the single natural_log_exp table set
    (one table load; sqrt via exp(0.5*ln)).
"""
import math

import numpy as np
import ml_dtypes

import concourse.bass as bass
import concourse.tile as tile
from concourse import bacc, mybir
from concourse.bass_utils import run_bass_kernel_spmd

B = 2048          # batch
D = 512           # feature dim
C = 32768         # classes
NCORES = 8
CS = C // NCORES  # 4096 classes per core
S = 64.0
MARGIN = 0.5
EPS = 1e-7
COSM = math.cos(MARGIN)
SINM = math.sin(MARGIN)

NB = B // 128     # 16 batch tiles
NK = D // 128     # 4 contraction chunks
NCC = CS // 512   # 8 class chunks per core
NH = NB // 2      # AllGather half

F32 = mybir.dt.float32
BF16 = mybir.dt.bfloat16
AF = mybir.ActivationFunctionType
ALU = mybir.AluOpType
BF16NP = ml_dtypes.bfloat16
FP8 = mybir.dt.float8e4
FP8NP = ml_dtypes.float8_e4m3fn
DR = mybir.MatmulPerfMode.DoubleRow

_CACHE = {}

_ONE_SET = "natural_log_exp_and_others"


def _patch_act_tables():
    from concourse import hw_specs, bacc as bacc_mod
    if getattr(bacc_mod, "_act_tables_patched", False):
        return
    orig = hw_specs.get_activation_tables

    def patched(arch):
        t = orig(arch)
        return {name: (funcs if name == _ONE_SET else set())
                for name, funcs in t.items()}

    bacc_mod.get_activation_tables = patched
    bacc_mod._act_tables_patched = True


def _build():
    _patch_act_tables()
    nc = bacc.Bacc(None, target_bir_lowering=False, debug=False)

    fT_ext = nc.declare_dram_parameter("fT", [D, B], FP8, isOutput=False)
    wT_ext = nc.declare_dram_parameter("wT", [D, CS], BF16, isOutput=False)
    fnat_ext = nc.declare_dram_parameter("fnat", [B, D], BF16, isOutput=False)
    wtgt_ext = nc.declare_dram_parameter("wtgt", [B, D], BF16, isOutput=False)
    out_ext = nc.declare_dram_parameter("out", [1, 1], F32, isOutput=True)

    ccA_in = nc.dram_tensor("ccA_in", [128, NB], F32)
    ccA_out = nc.dram_tensor("ccA_out", [128 * NCORES, NB], F32,
                             addr_space="Shared")
    ccB0_in = nc.dram_tensor("ccB0_in", [128, NH], F32)
    ccB0_out = nc.dram_tensor("ccB0_out", [128 * NCORES, NH], F32,
                              addr_space="Shared")
    ccB1_in = nc.dram_tensor("ccB1_in", [128, NH], F32)
    ccB1_out = nc.dram_tensor("ccB1_out", [128 * NCORES, NH], F32,
                              addr_space="Shared")

    with tile.TileContext(nc) as tc:
        with (
            tc.tile_pool(name="persist", bufs=1) as pp,
            tc.tile_pool(name="stream", bufs=4) as sp,
        ):
            # ---- persistent SBUF tiles ----
            wt3 = pp.tile([128, NK, CS], BF16)     # raw wT (bf16)
            whats = [pp.tile([128, NK, 512], FP8, tag=f"what{i}",
                             name=f"what{i}")
                     for i in range(NCC)]          # normalized wT, per chunk
            ft3 = pp.tile([128, NK, B], FP8)       # raw fT (fp8) = stationary
            fnat3 = pp.tile([128, NB, D], BF16)    # features, natural layout
            wtgt3 = pp.tile([128, NB, D], BF16)    # target weight rows
            ones128 = pp.tile([128, 128], BF16)
            ones_f32 = pp.tile([128, 1], F32)
            rnr_sb = pp.tile([128, NCC, 512], BF16)  # 1/||w_c||, replicated
            lnr_sb = pp.tile([128, 2048], F32)       # ln scratch
            ejunk = pp.tile([128, 2048], BF16)       # Exp write target
            sumsA = pp.tile([128, NB], F32)        # exp sums, chunks 0-3
            sumsB = pp.tile([128, NB], F32)        # exp sums, chunks 4-7
            rs_pt = pp.tile([128, NB], F32)        # 64/||f_b|| per-partition
            ssf = pp.tile([128, NB], F32)
            rawdot = pp.tile([128, NB], F32)
            wn2 = pp.tile([128, NB], F32)

            # ---- DMAs: batched, priority-ordered ----
            wTr = wT_ext[:].rearrange("(k p) c -> p k c", p=128)
            fTr = fT_ext[:].rearrange("(k p) b -> p k b", p=128)
            fnr = fnat_ext[:].rearrange("(t p) d -> p t d", p=128)
            wgr = wtgt_ext[:].rearrange("(t p) d -> p t d", p=128)
            # both hwdge queues (sync + scalar), alternating ~1KB-row
            # transfers: doubles descriptor-generation rate while staying
            # fine-grained enough for fair cross-core HBM arbitration
            # (2KB+ rows measured ~20us of extra peer skew at the first
            # AllGather — the laggard core dominates the metric, so
            # fairness beats local arrival time).
            for n in range(NCC):
                for kp in range(2):
                    eng = nc.sync if (2 * n + kp) % 2 == 0 else nc.scalar
                    eng.dma_start(
                        wt3[:, 2 * kp:2 * kp + 2, bass.ts(n, 512)],
                        wTr[:, 2 * kp:2 * kp + 2, bass.ts(n, 512)])
            for k in range(NK):
                eng = nc.sync if k % 2 == 0 else nc.scalar
                eng.dma_start(ft3[:, k, :], fTr[:, k, :])
            # fnat: fine-grained early tiles (ssf(t) feeds the Exp scale)
            for i, (t0, t1) in enumerate(
                    ((0, 1), (1, 2), (2, 4), (4, 8), (8, 12), (12, 16))):
                eng = nc.sync if i % 2 == 0 else nc.scalar
                eng.dma_start(fnat3[:, t0:t1, :], fnr[:, t0:t1, :])
            for i, t0 in enumerate(range(0, NB, 4)):
                eng = nc.sync if i % 2 == 0 else nc.scalar
                eng.dma_start(wtgt3[:, t0:t0 + 4, :],
                              wgr[:, t0:t0 + 4, :])

            nc.vector.memset(ones128[:], 1.0)
            nc.vector.memset(ones_f32[:], 1.0)

            pmain_cm = tc.tile_pool(name="pmain", bufs=2, space="PSUM")
            pmain = pmain_cm.__enter__()

            # ---- weight-column norms, both groups upfront ----
            nps = []
            for g in range(2):
                zs = pmain.tile([128, 2048], F32, tag="z", name=f"nps{g}")
                nps.append(zs)
                for c4 in range(4):
                    cc = 4 * g + c4
                    for k in range(NK):
                        sq = sp.tile([128, 512], BF16, tag="sqt", name="sq")
                        nc.vector.tensor_mul(sq[:],
                                             wt3[:, k, bass.ts(cc, 512)],
                                             wt3[:, k, bass.ts(cc, 512)])
                        nc.tensor.matmul(zs[:, bass.ts(c4, 512)],
                                         ones128[:], sq[:],
                                         start=(k == 0), stop=(k == NK - 1))
                # rnr = exp(-0.5*ln(nrm2)) on [128,1024] halves
                for h in range(2):
                    seg = zs[:, h * 1024:(h + 1) * 1024]
                    lseg = lnr_sb[:, h * 1024:(h + 1) * 1024]
                    nc.scalar.activation(lseg, seg, AF.Ln)
                    nc.scalar.activation(
                        rnr_sb[:, 4 * g + 2 * h:4 * g + 2 * h + 2, :]
                        .rearrange("p a b -> p (a b)"),
                        lseg, AF.Exp, scale=-0.5)
                for c4 in range(4):
                    cc = 4 * g + c4
                    for k in range(NK):
                        nc.vector.tensor_mul(whats[cc][:, k, :],
                                             wt3[:, k, bass.ts(cc, 512)],
                                             rnr_sb[:, cc, :])

            # LDWEIGHTS filler: the PE would otherwise sit idle for the
            # ~10us rnr/whats window, long enough for the HAM clock gate
            # to re-throttle it to 1.2GHz before the sweep even starts.
            # Standalone weight loads keep the array active (no PSUM
            # writes, no correctness impact — every real matmul reloads
            # its own stationary anyway).
            for _ in range(80):
                nc.tensor.ldweights(ones128[:])

            # ---- Pool muls + DVE reduces: ssf first, then rawdot/wn2 ----
            for t in range(NB):
                sqf = sp.tile([128, D], BF16, tag="prod", name="sqf")
                nc.gpsimd.tensor_mul(sqf[:], fnat3[:, t, :], fnat3[:, t, :])
                nc.vector.reduce_sum(ssf[:, t:t + 1], sqf[:],
                                     axis=mybir.AxisListType.X)

            def tgt_dots(t0, t1):
                for t in range(t0, t1):
                    prod = sp.tile([128, D], BF16, tag="prod", name="prod")
                    nc.gpsimd.tensor_mul(prod[:], fnat3[:, t, :],
                                         wtgt3[:, t, :])
                    nc.vector.reduce_sum(rawdot[:, t:t + 1], prod[:],
                                         axis=mybir.AxisListType.X)
                    sq2 = sp.tile([128, D], BF16, tag="prod", name="sq2")
                    nc.gpsimd.tensor_mul(sq2[:], wtgt3[:, t, :],
                                         wtgt3[:, t, :])
                    nc.vector.reduce_sum(wn2[:, t:t + 1], sq2[:],
                                         axis=mybir.AxisListType.X)

            tgt_dots(0, NB // 2)

            # rs = 64/||f|| = exp(-0.5*ln(ssf/4096)), batches of 4 b-tiles
            for h in range(0, NB, 4):
                lcol = sp.tile([128, 4], F32, tag="lcol", name="lcol")
                nc.scalar.activation(lcol[:], ssf[:, h:h + 4], AF.Ln,
                                     scale=1.0 / 4096.0)
                nc.scalar.activation(rs_pt[:, h:h + 4], lcol[:], AF.Exp,
                                     scale=-0.5)

            # ---- main sweeps: class-half-major, j-outer ----
            for g, sums in ((0, sumsA), (1, sumsB)):
                for b in range(NB):
                    zp = pmain.tile([128, 2048], F32, tag="z", name="zp")
                    for j in range(2):
                        for c4 in range(4):
                            cc = 4 * g + c4
                            nc.tensor.matmul(
                                zp[:, bass.ts(c4, 512)],
                                ft3[:, 2 * j:2 * j + 2, bass.ts(b, 128)],
                                whats[cc][:, 2 * j:2 * j + 2, :],
                                start=(j == 0), stop=(j == 1),
                                perf_mode=DR)
                    nc.scalar.activation(
                        ejunk[:], zp[:], AF.Exp, scale=rs_pt[:, b:b + 1],
                        accum_out=sums[:, b:b + 1])
                if g == 0:
                    nc.sync.dma_start(ccA_in[:], sumsA[:])
                    nc.gpsimd.collective_compute(
                        "AllGather", ALU.bypass,
                        replica_groups=[list(range(NCORES))],
                        ins=[ccA_in[:].opt()],
                        outs=[ccA_out[:].opt()],
                    )
                    tgt_dots(NB // 2, NB)

            # sumsB halves: b0-7 fires at EXP-B(7); b8-15 rides the tail
            nc.sync.dma_start(ccB0_in[:], sumsB[:, 0:NH])
            nc.gpsimd.collective_compute(
                "AllGather", ALU.bypass,
                replica_groups=[list(range(NCORES))],
                ins=[ccB0_in[:].opt()],
                outs=[ccB0_out[:].opt()],
            )
            nc.sync.dma_start(ccB1_in[:], sumsB[:, NH:])
            nc.gpsimd.collective_compute(
                "AllGather", ALU.bypass,
                replica_groups=[list(range(NCORES))],
                ins=[ccB1_in[:].opt()],
                outs=[ccB1_out[:].opt()],
            )
            # gathers after all cc-input DMAs (no Sync FIFO head-of-line)
            gathA = pp.tile([128, NCORES, NB], F32)
            nc.sync.dma_start(
                gathA[:], ccA_out[:].rearrange("(g p) c -> p g c", p=128))
            fullsumA = pp.tile([128, NB], F32)
            nc.vector.tensor_reduce(
                fullsumA[:], gathA[:].rearrange("p g c -> p c g"),
                axis=mybir.AxisListType.X, op=ALU.add)
            gathB0 = pp.tile([128, NCORES, NH], F32)
            nc.sync.dma_start(
                gathB0[:], ccB0_out[:].rearrange("(g p) c -> p g c", p=128))
            gathB1 = pp.tile([128, NCORES, NH], F32)
            nc.sync.dma_start(
                gathB1[:], ccB1_out[:].rearrange("(g p) c -> p g c", p=128))
            fullsumB = pp.tile([128, NB], F32)
            nc.vector.tensor_reduce(
                fullsumB[:, 0:NH], gathB0[:].rearrange("p g c -> p c g"),
                axis=mybir.AxisListType.X, op=ALU.add)
            nc.vector.tensor_reduce(
                fullsumB[:, NH:], gathB1[:].rearrange("p g c -> p c g"),
                axis=mybir.AxisListType.X, op=ALU.add)

            # ---- combine: full-width parts (hidden under the sweep) ----
            m2 = pp.tile([128, NB], F32)
            nc.vector.tensor_mul(m2[:], ssf[:], wn2[:])
            lm2 = pp.tile([128, NB], F32)
            nc.scalar.activation(lm2[:], m2[:], AF.Ln)
            rboth = pp.tile([128, NB], F32)
            nc.scalar.activation(rboth[:], lm2[:], AF.Exp, scale=-0.5)
            tgt = pp.tile([128, NB], F32)
            nc.vector.tensor_mul(tgt[:], rawdot[:], rboth[:])
            exptgt = pp.tile([128, NB], F32)
            nc.scalar.activation(exptgt[:], tgt[:], AF.Exp, scale=S)
            tclip = pp.tile([128, NB], F32)
            nc.vector.tensor_scalar(
                tclip[:], tgt[:], -1.0 + EPS, 1.0 - EPS,
                op0=ALU.max, op1=ALU.min)
            om = pp.tile([128, NB], F32)
            nc.vector.tensor_mul(om[:], tclip[:], tclip[:])
            nc.vector.tensor_scalar(om[:], om[:], -1.0, 1.0,
                                    op0=ALU.mult, op1=ALU.add)
            lom = pp.tile([128, NB], F32)
            nc.scalar.activation(lom[:], om[:], AF.Ln)
            snt = pp.tile([128, NB], F32)
            nc.scalar.activation(snt[:], lom[:], AF.Exp, scale=0.5)
            num = pp.tile([128, NB], F32)
            nc.vector.tensor_scalar_mul(num[:], tclip[:], S * COSM)
            snts = pp.tile([128, NB], F32)
            nc.vector.tensor_scalar_mul(snts[:], snt[:], S * SINM)
            nc.vector.tensor_sub(num[:], num[:], snts[:])
            expnum = pp.tile([128, NB], F32)
            nc.scalar.activation(expnum[:], num[:], AF.Exp)

            # ---- denominator chain, split in AllGather halves ----
            fullsum = pp.tile([128, NB], F32)
            denom = pp.tile([128, NB], F32)
            logd = pp.tile([128, NB], F32)
            lvals = pp.tile([128, NB], F32)
            lreds = pp.tile([128, 2], F32)
            for h in range(2):
                sl = slice(h * NH, (h + 1) * NH)
                nc.vector.tensor_add(fullsum[:, sl], fullsumA[:, sl],
                                     fullsumB[:, sl])
                nc.vector.tensor_add(denom[:, sl], expnum[:, sl],
                                     fullsum[:, sl])
                nc.vector.tensor_sub(denom[:, sl], denom[:, sl],
                                     exptgt[:, sl])
                nc.scalar.activation(logd[:, sl], denom[:, sl], AF.Ln)
                nc.vector.tensor_sub(lvals[:, sl], num[:, sl], logd[:, sl])
                nc.vector.reduce_sum(lreds[:, h:h + 1], lvals[:, sl],
                                     axis=mybir.AxisListType.X)
            lred = pp.tile([128, 1], F32)
            nc.vector.tensor_add(lred[:], lreds[:, 0:1], lreds[:, 1:2])
            zf = pmain.tile([128, 2048], F32, tag="z", name="zf")
            nc.tensor.matmul(zf[0:1, 0:1], ones_f32[:], lred[:],
                             start=True, stop=True)
            outv = pp.tile([1, 1], F32)
            nc.scalar.mul(outv[:], zf[0:1, 0:1], -1.0 / float(B))
            nc.sync.dma_start(out_ext[:], outv[:])
            pmain_cm.__exit__(None, None, None)

    nc.compile()
    return nc


def _prep_inputs(features, y_true, weight):
    features = np.asarray(features, dtype=np.float32)
    weight = np.asarray(weight, dtype=np.float32)
    y = np.asarray(y_true).astype(np.int64)

    fT = features.T.astype(FP8NP, order="C")           # [D, B]
    fnat = features.astype(BF16NP)                     # [B, D] bf16
    wtgt = weight[y].astype(BF16NP)                    # [B, D] bf16

    in_maps = []
    for i in range(NCORES):
        shard = weight[i * CS:(i + 1) * CS]            # [CS, D]
        wT = shard.T.astype(BF16NP, order="C")         # [D, CS]
        in_maps.append({"fT": fT, "wT": wT, "fnat": fnat, "wtgt": wtgt})
    return in_maps


def _run(features, y_true, weight, trace=False, **run_kwargs):
    if "nc" not in _CACHE:
        _CACHE["nc"] = _build()
    nc = _CACHE["nc"]
    in_maps = _prep_inputs(features, y_true, weight)
    res = run_bass_kernel_spmd(
        nc, in_maps, core_ids=list(range(NCORES)), trace=trace, **run_kwargs)
    out = np.asarray(res.results[0]["out"], dtype=np.float32)
    return np.float32(out.reshape(-1)[0]), res


def kernel(features, y_true, weight):
    val, _ = _run(features, y_true, weight, trace=False)
    return np.asarray(val, dtype=np.float32)
